# revision 98
# baseline (speedup 1.0000x reference)
"""Trainium2 Bass kernel for nn_DSQGAttentionN (sparse offset-attention).

Sharding: 16 heads / 8 cores = 2 heads per core (head parallel). Each core
computes its 2 heads' attention plus a column-shard of the gate and output
projection, producing a full-shape (2048, 1024) partial output; the host sums
the 8 partials and adds the output bias.

v4 design:
- Error-compensated fp8 DoubleRow projections: x = x8 + dx8/XDSC, W = W8 + dW8
  (both W terms share the x64 scale), so each projection is 3 DR matmul sets
  (x8*W8 + dx8*W8/XDSC + x8*dW8) with bf16-level accuracy at DR speed.
- V projection emitted directly in [n, j] layout (x chunk stationary, Wv
  moving); bias + softmax-z ones columns ride a rank-1 matmul.
- Dense band offsets 0..192 via 320-wide score windows; exp batched over
  both heads from a [128,2,320] PSUM tile; em mask on DVE/Pool per 4-chunk
  group.
- Dyadic offsets (256..1536): shifted q*k products (validity trimmed),
  ones-matmul reductions, then per-group AV terms as broadcast muls into
  SBUF scratch which the PE accumulates into the group od PSUM via
  identity matmuls (adds run on PE, not DVE). zdy is also matmul-added.
- Pass 2 pipelined one 4-chunk group behind the band; od+dyadic+z live in
  one group PSUM tile (odg); normalization drains on Act with per-partition
  reciprocal scale; out-proj bf16; outsb drains split DVE/Act; DMA out per
  2 chunks.
"""

import math

import numpy as np
import ml_dtypes

BF16 = ml_dtypes.bfloat16
F8 = ml_dtypes.float8_e4m3

N, D, H, HD = 2048, 1024, 16, 64
NCH = 16
NC2 = 4               # DR contraction groups (256 features each)
W = 320
DY6 = [256, 384, 512, 768, 1024, 1536]
NDY = 6
VPAD = 3
NCORES = 8
WSCALE = 64.0         # fp8 weight pre-scale

_DENSE_LOCAL_W = 64
_DYADIC = [96, 128, 192, 256, 384, 512, 768, 1024, 1536]
OFFSETS = sorted(set(range(0, _DENSE_LOCAL_W + 1)) | set(_DYADIC))
BAND_OFFS = sorted(set(range(0, 65)) | {96, 128, 192})

# layout of the packed bf16 tensor (per-partition columns)
_BF_WO = 0
_BF_EM = _BF_WO + D               # [2, W]
_BF_V9 = _BF_EM + 2 * W           # [NCH, NDY, 2]
_BF_END = _BF_V9 + NCH * NDY * 2

_STATE = {}


def _build_nc(debug=False):
    import concourse.bass as bass
    import concourse.tile as tile
    import concourse.mybir as mybir
    from concourse import bacc
    from concourse.bass import ds
    from concourse.masks import make_identity
    from contextlib import ExitStack

    dt = mybir.dt
    f32, bf, f8 = dt.float32, dt.bfloat16, dt.float8e4
    AF = mybir.ActivationFunctionType
    OP = mybir.AluOpType
    AX = mybir.AxisListType
    PM = mybir.MatmulPerfMode

    nc = bacc.Bacc("TRN2")

    # DRAM I/O
    xdr_d = [nc.dram_tensor(f"xdr{c}", (128, 2, N), f8, kind="ExternalInput").ap()
             for c in range(NC2)]
    xde_d = [nc.dram_tensor(f"xde{c}", (128, 2, N), f8, kind="ExternalInput").ap()
             for c in range(NC2)]
    # w8all: [wq, wk, wg, dwq, dwk, dwg] each [NC2,2,128] then [wv, dwv] [NC2,2,130]
    w8_d = nc.dram_tensor("w8all", (128, 6 * NC2 * 2 * 128 + 2 * NC2 * 2 * 130),
                          f8, kind="ExternalInput").ap()
    bfp_d = nc.dram_tensor("bfall", (128, _BF_END), bf, kind="ExternalInput").ap()
    bias_d = nc.dram_tensor("bias4", (128, 4), f32, kind="ExternalInput").ap()
    aux_d = nc.dram_tensor("aux", (1, 384), bf, kind="ExternalInput").ap()
    out = nc.dram_tensor("out", (128, NCH, D), bf, kind="ExternalOutput").ap()
    dbg = {}
    if debug:
        for nm, shp, dtt in [
            ("dqt", (128, N), bf), ("dkt", (128, N), bf),
            ("dgt", (128, N), bf), ("dv2", (128, VPAD + NCH, 130), bf),
            ("dpt", (128, 2, NCH, W), bf), ("dzdy", (128, NCH, 2), bf),
            ("ddyPh", (128, NCH, NDY, 2), bf), ("dfgT", (128, NCH, 128), bf),
            ("dat", (128, 2, 128), bf), ("drz", (128, 2, 2), dt.float32),
            ("dodg", (128, 2, 130), dt.float32),
        ]:
            dbg[nm] = nc.dram_tensor(nm, shp, dtt, kind="ExternalOutput").ap()

    with tile.TileContext(nc) as tc, ExitStack() as ctx:
        sing = ctx.enter_context(tc.tile_pool(name="sing", bufs=1))

        xdr = [sing.tile([128, 2, N], f8, name=f"xdr{c}") for c in range(NC2)]
        xde = [sing.tile([128, 2, N], f8, name=f"xde{c}") for c in range(NC2)]
        w8 = sing.tile([128, 6 * NC2 * 2 * 128 + 2 * NC2 * 2 * 130], f8)
        bfp = sing.tile([128, _BF_END], bf)
        bias_s = sing.tile([128, 4], f32)
        aux_s = sing.tile([1, 384], bf)

        SEG = NC2 * 2 * 128
        wq8 = w8[:, 0 * SEG:1 * SEG].rearrange("p (c r m) -> p c r m", c=NC2, r=2)
        wk8 = w8[:, 1 * SEG:2 * SEG].rearrange("p (c r m) -> p c r m", c=NC2, r=2)
        wg8 = w8[:, 2 * SEG:3 * SEG].rearrange("p (c r m) -> p c r m", c=NC2, r=2)
        dwq8 = w8[:, 3 * SEG:4 * SEG].rearrange("p (c r m) -> p c r m", c=NC2, r=2)
        dwk8 = w8[:, 4 * SEG:5 * SEG].rearrange("p (c r m) -> p c r m", c=NC2, r=2)
        dwg8 = w8[:, 5 * SEG:6 * SEG].rearrange("p (c r m) -> p c r m", c=NC2, r=2)
        SEGV = NC2 * 2 * 130
        wv8 = w8[:, 6 * SEG:6 * SEG + SEGV].rearrange(
            "p (c r m) -> p c r m", c=NC2, r=2)
        dwv8 = w8[:, 6 * SEG + SEGV:6 * SEG + 2 * SEGV].rearrange(
            "p (c r m) -> p c r m", c=NC2, r=2)

        wo_s = bfp[:, _BF_WO:_BF_EM]
        em_s = bfp[:, _BF_EM:_BF_V9].rearrange("p (h w) -> p h w", h=2)
        v9_s = bfp[:, _BF_V9:_BF_END].rearrange(
            "p (c i h) -> p c i h", c=NCH, i=NDY)

        qt = sing.tile([128, N], bf)
        kt = sing.tile([128, N], bf)
        gt = sing.tile([128, N], bf)
        v2i = sing.tile([128, VPAD + NCH, 130], bf)
        prod = sing.tile([128, NDY, N], bf)
        ptall = sing.tile([128, 2, NCH, W], bf)
        dyPh = sing.tile([128, NCH, NDY, 2], bf)
        zdy = sing.tile([128, NCH, 2], bf)
        dyA = sing.tile([128, NCH, 128], bf)
        fgT = sing.tile([128, NCH, 128], bf)
        outsb = sing.tile([128, NCH, D], bf)
        hmask = sing.tile([128, 2], bf)
        ident = sing.tile([128, 128], bf)
        dbg_at = None
        dbg_rz = None
        if debug:
            dbg_at = sing.tile([128, 2, 128], bf, name="dbg_at")
            dbg_rz = sing.tile([128, 2, 2], f32, name="dbg_rz")
            dbg_odg = sing.tile([128, 2, 130], f32, name="dbg_odg")

        bq_c = bias_s[:, 0:1]
        bk_c = bias_s[:, 1:2]
        bg_c = bias_s[:, 2:3]

        bvrow = aux_s[:, 0:130]
        ones1 = aux_s[:, 130:258]

        def ap_of(t, extra, off_elems=0):
            return bass.AP(
                tensor=t.tensor, offset=t.offset + off_elems,
                ap=[list(t.ap[0])] + extra,
            )

        make_identity(nc, ident)
        nc.gpsimd.memset(hmask, 0.0)
        nc.gpsimd.memset(hmask[0:64, 0:1], 1.0)
        nc.gpsimd.memset(hmask[64:128, 1:2], 1.0)
        nc.gpsimd.memset(v2i[:, 0:VPAD, :], 0.0)
        nc.gpsimd.memset(dyA[:, 0:2, :], 0.0)

        # ---- input DMAs (order matters: HWDGE serializes) ----
        nc.sync.dma_start(out=xdr[0], in_=xdr_d[0])
        nc.sync.dma_start(out=w8, in_=w8_d)
        nc.sync.dma_start(out=xde[0], in_=xde_d[0])
        nc.sync.dma_start(out=bias_s, in_=bias_d)
        nc.sync.dma_start(out=aux_s, in_=aux_d)
        for c2 in range(1, NC2):
            nc.sync.dma_start(out=xdr[c2], in_=xdr_d[c2])
            nc.sync.dma_start(out=xde[c2], in_=xde_d[c2])
        nc.sync.dma_start(out=bfp, in_=bfp_d)

        # ================= QK projection (compensated fp8 DR) ============
        with ExitStack() as pctx:
            psQK = pctx.enter_context(
                tc.tile_pool(name="psQK", bufs=8, space="PSUM")
            )
            qk_ps = [
                psQK.tile([128, 512], f32, tag="qk", name=f"qk{i}")
                for i in range(8)
            ]
            for wu in range(30):
                nc.tensor.matmul(
                    qk_ps[7][:, 0:128], ident, ident,
                    start=True, stop=True, skip_group_check=True,
                )
            # all 3 comp sets per c2: PE streams behind the per-c2 DMAs
            for c2 in range(NC2):
                for xs, w_list in ((xdr, (wq8, wk8)), (xdr, (dwq8, dwk8)),
                                   (xde, (wq8, wk8))):
                    for pj, w_sb in enumerate(w_list):
                        for b in range(4):
                            nc.tensor.matmul(
                                qk_ps[pj * 4 + b], w_sb[:, c2],
                                xs[c2][:, :, ds(b * 512, 512)],
                                start=(c2 == 0 and xs is xdr
                                       and w_sb in (wq8, wk8)
                                       and w_list[0] is wq8),
                                stop=(c2 == NC2 - 1 and xs is xde),
                                perf_mode=PM.DoubleRow,
                            )
            sq = 1.0 / (WSCALE * 8.0)
            sk = 1.0 / WSCALE
            nc.scalar.activation(qt[:, ds(0, 512)], qk_ps[0], AF.Identity,
                                 bias=bq_c, scale=sq)
            nc.vector.tensor_scalar(kt[:, ds(0, 512)], qk_ps[4],
                                    sk, bk_c, OP.mult, OP.add)
            nc.scalar.activation(qt[:, ds(512, 512)], qk_ps[1], AF.Identity,
                                 bias=bq_c, scale=sq)
            nc.vector.tensor_scalar(kt[:, ds(512, 512)], qk_ps[5],
                                    sk, bk_c, OP.mult, OP.add)
            nc.scalar.activation(qt[:, ds(1024, 512)], qk_ps[2], AF.Identity,
                                 bias=bq_c, scale=sq)
            nc.vector.tensor_scalar(kt[:, ds(1024, 512)], qk_ps[6],
                                    sk, bk_c, OP.mult, OP.add)
            nc.vector.tensor_scalar(qt[:, ds(1536, 512)], qk_ps[3],
                                    sq, bq_c, OP.mult, OP.add)
            nc.scalar.activation(kt[:, ds(1536, 512)], qk_ps[7], AF.Identity,
                                 bias=bk_c, scale=sk)

        # ================= middle phase =================
        with ExitStack() as mctx:
            psDY = mctx.enter_context(
                tc.tile_pool(name="psDY", bufs=1, space="PSUM"))
            sbp = mctx.enter_context(tc.tile_pool(name="sbp", bufs=2))

            dy_ps = psDY.tile([128, NCH, NDY * 2], f32, name="dy_ps")
            nc.vector.memset(dy_ps, 0.0)

            def emit_prod(i):
                d = DY6[i]
                nc.vector.tensor_mul(
                    prod[:, i, d:N], qt[:, d:N], kt[:, 0:N - d]
                )

            def emit_reduce(i):
                m = DY6[i] // 128
                for c in range(m, NCH):
                    o1 = bass.AP(
                        tensor=dy_ps.tensor,
                        offset=dy_ps.offset + c * NDY * 2 + i * 2,
                        ap=[list(dy_ps.ap[0]), [1, 2]],
                    )
                    nc.tensor.matmul(
                        o1, prod[:, i, ds(c * 128, 128)], hmask,
                        start=True, stop=True, skip_group_check=True,
                    )

            odg_tiles = {}

            def valid_pair(p):
                return [i for i in range(NDY) if DY6[i] // 128 <= 2 * p + 1]

            def emit_od_pair(p):
                odg = psO.tile([128, 2, 130], f32, tag="odg", name="odg")
                odg_tiles[p] = odg
                for u in range(2):
                    qc = 2 * p + u
                    nsub = 1 + (qc >= 1) + (qc >= 2)
                    for hl in range(2):
                        mv = ap_of(v2i, [[2, 65]], (VPAD + qc) * 130 + hl)
                        nc.tensor.matmul(
                            ap_of(odg, [[1, 65]], u * 130 + hl * 65),
                            ptall[:, hl, qc, 0:128], mv,
                            start=True, stop=(nsub == 1),
                            skip_group_check=True,
                        )
                        if qc >= 1:
                            mv = ap_of(v2i, [[2, 65]],
                                       (VPAD + qc - 1) * 130 + hl)
                            nc.tensor.matmul(
                                ap_of(odg, [[1, 65]], u * 130 + hl * 65),
                                ptall[:, hl, qc - 1, 128:256], mv,
                                start=False, stop=(nsub == 2),
                                skip_group_check=True,
                            )
                        if qc >= 2:
                            mv = ap_of(v2i, [[2, 65]],
                                       (VPAD + qc - 2) * 130 + hl)
                            nc.tensor.matmul(
                                bass.AP(
                                    tensor=odg.tensor,
                                    offset=odg.offset + u * 130 + hl * 65,
                                    ap=[[list(odg.ap[0])[0], 64], [1, 65]],
                                ),
                                ptall[:, hl, qc - 2, 256:W], mv,
                                start=False, stop=True,
                                skip_group_check=True,
                            )
            def av_pair(p):
                # dyadic AV accumulation into dyA (DVE for late pairs, Pool
                # for early ones which land while DVE is still busy)
                valid = valid_pair(p)
                eng2 = nc.gpsimd if p <= 3 else nc.vector

                def acc_v():
                    return ap_of(dyA, [[128, 2], [2, 64], [1, 2]],
                                 2 * p * 128)

                def src_v(i):
                    m = DY6[i] // 128
                    return ap_of(v2i, [[130, 2], [2, 64], [1, 2]],
                                 (VPAD - m + 2 * p) * 130)

                def alpha_v(i):
                    return ap_of(dyPh, [[NDY * 2, 2], [0, 64], [1, 2]],
                                 2 * p * NDY * 2 + i * 2)

                eng3 = eng2
                if valid:
                    eng2.tensor_mul(acc_v(), src_v(valid[0]),
                                    alpha_v(valid[0]))
                    for i in valid[1:]:
                        tmp = sbp.tile([128, 2, 128], bf, tag="avt",
                                       name="avt", bufs=4)
                        tv = ap_of(tmp, [[128, 2], [2, 64], [1, 2]])
                        eng2.tensor_mul(tv, src_v(i), alpha_v(i))
                        eng3.tensor_add(acc_v(), acc_v(), tv)

            at_tiles = {}

            ods_tiles = {}

            def emit_ods(p):
                odg = odg_tiles.pop(p)
                if debug and p == 0:
                    nc.vector.tensor_copy(dbg_odg, odg)
                ods = sbf.tile([128, 2, 130], bf, tag="ods", name="ods",
                               bufs=4)
                nc.scalar.copy(ods, odg)
                ods_tiles[p] = (ods, odg)

            def pass2_stage1(p):
                ods, odg = ods_tiles.pop(p)
                ats = []
                for u in range(2):
                    qc = 2 * p + u
                    zt = sbf.tile([128, 2], f32, tag="zt", name="zt")
                    nc.vector.tensor_add(
                        zt, ap_of(odg, [[65, 2]], u * 130 + 64),
                        zdy[:, qc, :])
                    rzt = sbf.tile([128, 2], f32, tag="rzt", name="rzt")
                    nc.vector.reciprocal(rzt, zt)
                    a_t = sbf.tile([128, 128], bf, tag="a", name="a", bufs=16)
                    av = ap_of(a_t, [[2, 64], [1, 2]])
                    nc.vector.tensor_add(
                        av,
                        ap_of(ods, [[1, 64], [65, 2]], u * 130),
                        ap_of(dyA, [[2, 64], [1, 2]], qc * 128),
                    )
                    for hl in range(2):
                        nc.vector.tensor_scalar_mul(
                            ap_of(a_t, [[2, 64]], hl),
                            ap_of(a_t, [[2, 64]], hl),
                            rzt[:, hl:hl + 1],
                        )
                    if debug and qc in (0, 1):
                        nc.vector.tensor_copy(dbg_at[:, qc, :], a_t)
                        nc.vector.tensor_copy(dbg_rz[:, qc, :], rzt)
                    ats.append(a_t)
                at_tiles[p] = ats


            # ---- scope A: projections + bands + dyadic prep ----
            with ExitStack() as vctx:
                psV = vctx.enter_context(
                    tc.tile_pool(name="psV", bufs=2, space="PSUM"))
                psG = vctx.enter_context(
                    tc.tile_pool(name="psG", bufs=2, space="PSUM"))
                psS = vctx.enter_context(
                    tc.tile_pool(name="psS", bufs=2, space="PSUM"))

                def band_kc(kc):
                    span = min(W, N - 128 * kc)
                    for hl in range(2):
                        st = psS.tile([128, W], f32, tag="st", name="st")
                        hp = ds(64 * hl, 64)
                        nc.tensor.matmul(
                            st[:, 0:span],
                            kt[hp, ds(kc * 128, 128)],
                            qt[hp, ds(kc * 128, span)],
                            start=True, stop=True, skip_group_check=True,
                        )
                        nc.scalar.activation(
                            ptall[:, hl, kc, 0:span], st[:, 0:span], AF.Exp
                        )

                def band_mask(g):
                    o = ap_of(ptall, [[NCH * W, 2], [W, 4], [1, W]], g * 4 * W)
                    e = ap_of(em_s, [[W, 2], [0, 4], [1, W]])
                    nc.vector.tensor_mul(o, o, e)

                def dyadic_group(g):
                    g4 = ds(g * 4, 4)
                    nc.scalar.activation(dyPh[:, g4, :, :], dy_ps[:, g4, :],
                                         AF.Exp)
                    eng = nc.gpsimd if g == 0 else nc.vector
                    eng.tensor_mul(dyPh[:, g4, :, :], dyPh[:, g4, :, :],
                                   v9_s[:, g4, :, :])
                    with nc.allow_low_precision(reason="z of <=6 bf16 terms"):
                        nc.vector.tensor_reduce(
                            zdy[:, g4, :],
                            dyPh[:, g4, :, :].rearrange("p c i h -> p c h i"),
                            AX.X, OP.add,
                        )

                def v_pair(c0):
                    ps = psV.tile([128, 2, 130], f32, tag="v", name="v")
                    for u in range(2):
                        c = c0 + u
                        for c2 in range(NC2):
                            nc.tensor.matmul(
                                ps[:, u], xdr[c2][:, :, ds(c * 128, 128)],
                                wv8[:, c2],
                                start=(c2 == 0), stop=False,
                                perf_mode=PM.DoubleRow,
                                skip_group_check=True,
                            )
                        for c2 in range(NC2):
                            nc.tensor.matmul(
                                ps[:, u], xdr[c2][:, :, ds(c * 128, 128)],
                                dwv8[:, c2],
                                start=False, stop=False,
                                perf_mode=PM.DoubleRow,
                                skip_group_check=True,
                            )
                        for c2 in range(NC2):
                            nc.tensor.matmul(
                                ps[:, u], xde[c2][:, :, ds(c * 128, 128)],
                                wv8[:, c2],
                                start=False, stop=False,
                                perf_mode=PM.DoubleRow,
                                skip_group_check=True,
                            )
                        nc.tensor.matmul(ps[:, u], ones1, bvrow,
                                         start=False, stop=True,
                                         skip_group_check=True)
                    nc.vector.tensor_scalar_mul(
                        v2i[:, VPAD + c0:VPAD + c0 + 2, :], ps,
                        1.0 / WSCALE)

                def g_block(b):
                    ps = psG.tile([128, 512], f32, tag="g", name="g")
                    for c2 in range(NC2):
                        nc.tensor.matmul(
                            ps, wg8[:, c2], xdr[c2][:, :, ds(b * 512, 512)],
                            start=(c2 == 0), stop=False,
                            perf_mode=PM.DoubleRow,
                        )
                    for c2 in range(NC2):
                        nc.tensor.matmul(
                            ps, dwg8[:, c2], xdr[c2][:, :, ds(b * 512, 512)],
                            start=False, stop=False,
                            perf_mode=PM.DoubleRow,
                        )
                    for c2 in range(NC2):
                        nc.tensor.matmul(
                            ps, wg8[:, c2], xde[c2][:, :, ds(b * 512, 512)],
                            start=False, stop=(c2 == NC2 - 1),
                            perf_mode=PM.DoubleRow,
                        )
                    nc.scalar.activation(gt[:, ds(b * 512, 512)], ps, AF.Tanh,
                                         bias=bg_c, scale=0.5 / WSCALE)

                for kc in range(0, 4):
                    band_kc(kc)
                for c0 in range(0, 8, 2):
                    v_pair(c0)
                for kc in range(4, 8):
                    band_kc(kc)
                emit_prod(5)
                emit_prod(4)
                for kc in range(8, 12):
                    band_kc(kc)
                emit_prod(3)
                for c0 in range(8, NCH, 2):
                    v_pair(c0)
                emit_prod(2)
                for kc in range(12, NCH):
                    band_kc(kc)
                emit_prod(1)
                g_block(0)
                emit_prod(0)
                emit_reduce(5)
                g_block(1)
                emit_reduce(4)
                emit_reduce(3)
                band_mask(0)
                g_block(2)
                emit_reduce(2)
                band_mask(1)
                g_block(3)
                emit_reduce(1)
                emit_reduce(0)
                for b in range(4):
                    nc.vector.tensor_scalar(
                        gt[:, ds(b * 512, 512)], gt[:, ds(b * 512, 512)],
                        0.5, 0.5, OP.mult, OP.add,
                    )
                band_mask(2)
                band_mask(3)
                for g in range(4):
                    dyadic_group(g)
            # ---- scope B: od + pass 2 ----
            psO = mctx.enter_context(
                tc.tile_pool(name="psO", bufs=2, space="PSUM"))
            psF = mctx.enter_context(
                tc.tile_pool(name="psF", bufs=2, space="PSUM"))
            psQ = mctx.enter_context(
                tc.tile_pool(name="psQ", bufs=3, space="PSUM"))
            sbf = mctx.enter_context(tc.tile_pool(name="sbf", bufs=4))

            def pass2_stage2(p):
                ats = at_tiles.pop(p)
                for u in range(2):
                    qc = 2 * p + u
                    a_t = ats[u]
                    tp2 = psF.tile([128, 128], bf, tag="tp2", name="tp2")
                    nc.tensor.transpose(tp2, a_t, ident)
                    nc.vector.tensor_mul(fgT[:, qc, :], tp2,
                                         gt[:, ds(qc * 128, 128)])
                    for jb in range(2):
                        po = psQ.tile([128, 512], f32, tag="po", name="po")
                        nc.tensor.matmul(po, fgT[:, qc, :],
                                         wo_s[:, ds(jb * 512, 512)],
                                         start=True, stop=True)
                        osl = outsb[:, qc, ds(jb * 512, 512)]
                        if (2 * qc + jb) % 8 < 7:
                            nc.scalar.copy(osl, po)
                        else:
                            nc.vector.tensor_copy(osl, po)
                    nc.sync.dma_start(
                        out=out[:, qc:qc + 1, :],
                        in_=outsb[:, qc:qc + 1, :],
                    )

            for p in range(8):
                if p >= 1:
                    pass2_stage2(p - 1)
                emit_od_pair(p)
                av_pair(p)
                emit_ods(p)
                pass2_stage1(p)
            pass2_stage2(7)

            if debug:
                for nm, sb in [("dqt", qt), ("dkt", kt), ("dgt", gt),
                               ("dv2", v2i), ("dpt", ptall), ("dzdy", zdy),
                               ("ddyPh", dyPh), ("dfgT", fgT),
                               ("dat", dbg_at), ("drz", dbg_rz),
                               ("dodg", dbg_odg)]:
                    if nm in dbg:
                        nc.sync.dma_start(out=dbg[nm], in_=sb)

    nc.compile()
    return nc


def _host_prep(x, Wqkv, bqkv, Wgate, bgate, Wout, bout, pos_bias):
    x2 = np.asarray(x, np.float32).reshape(N, D)
    # xdr[p, c2, r, n] = x[n, 256*c2 + 128*r + p]
    xT = np.ascontiguousarray(x2.T).reshape(NC2, 2, 128, N)
    xTp = np.ascontiguousarray(xT.transpose(2, 0, 1, 3))
    xdr = xTp.astype(F8)
    xde = (xTp - xdr.astype(np.float32)).astype(F8)
    xmap = {}
    for c in range(NC2):
        xmap[f"xdr{c}"] = np.ascontiguousarray(xdr[:, c])
        xmap[f"xde{c}"] = np.ascontiguousarray(xde[:, c])

    psi = np.array([64 * (j % 2) + j // 2 for j in range(128)])

    def wpack8(Wcols):
        # Wcols: (1024, ncol) f32 *WSCALE -> base fp8 [p, c2, r, m] and delta
        ncol = Wcols.shape[1]
        Wt = np.ascontiguousarray(Wcols * WSCALE).reshape(NC2, 2, 128, ncol)
        Wt = np.ascontiguousarray(Wt.transpose(2, 0, 1, 3))
        w0 = Wt.astype(F8)
        dw = (Wt - w0.astype(np.float32)).astype(F8)
        return w0, dw

    in_maps = []
    for cid in range(NCORES):
        r0 = 128 * cid
        wq, dwq = wpack8(Wqkv[r0:r0 + 128].T)
        wk, dwk = wpack8(Wqkv[D + r0:D + r0 + 128].T)
        wgp, dwg = wpack8(Wgate[r0:r0 + 128][psi].T)
        wv130 = np.zeros((D, 130), np.float32)
        wv130[:, 0:128] = Wqkv[2 * D + r0:2 * D + r0 + 128][psi].T
        wv, dwv = wpack8(wv130)

        w8all = np.concatenate(
            [t.reshape(128, -1) for t in (wq, wk, wgp, dwq, dwk, dwg, wv, dwv)],
            axis=1,
        )

        wop = np.ascontiguousarray(Wout[:, r0:r0 + 128][:, psi].T)  # (128,1024)

        em = np.zeros((128, 2, W), np.float32)
        p = np.arange(128)[:, None]
        j = np.arange(W)[None, :]
        o = j - p
        sel = np.isin(o, BAND_OFFS)
        oc = np.clip(o, 0, 192)
        for hl in range(2):
            eb = np.array(
                [math.exp(pos_bias[OFFSETS.index(v), 2 * cid + hl])
                 if v in BAND_OFFS else 0.0 for v in range(193)],
                np.float32,
            )
            em[:, hl, :] = np.where(sel, eb[oc], 0.0)

        nidx = (np.arange(128)[:, None, None, None]
                + 128 * np.arange(NCH)[None, :, None, None])
        dys = np.array(DY6)[None, None, :, None]
        valid = (nidx >= dys).astype(np.float32)
        eb9 = np.zeros((1, 1, NDY, 2), np.float32)
        for hl in range(2):
            for i, d in enumerate(DY6):
                eb9[0, 0, i, hl] = math.exp(
                    pos_bias[OFFSETS.index(d), 2 * cid + hl])
        v9t = valid * eb9

        bfall = np.concatenate(
            [wop, em.reshape(128, -1), v9t.reshape(128, -1)], axis=1,
        ).astype(BF16)

        biases = np.stack(
            [
                bqkv[r0:r0 + 128] * 0.125,
                bqkv[D + r0:D + r0 + 128],
                0.5 * bgate[r0:r0 + 128][psi],
                np.zeros(128, np.float32),
            ],
            axis=1,
        ).astype(np.float32)

        aux = np.zeros((1, 384), np.float32)
        aux[0, 0:128] = bqkv[2 * D + r0:2 * D + r0 + 128][psi] * WSCALE
        aux[0, 128:130] = WSCALE
        aux[0, 130:258] = 1.0

        in_maps.append({
            **xmap,
            "w8all": np.ascontiguousarray(w8all),
            "bfall": np.ascontiguousarray(bfall),
            "bias4": np.ascontiguousarray(biases),
            "aux": np.ascontiguousarray(aux).astype(BF16),
        })
    return in_maps


def kernel(x, Wqkv, bqkv, Wgate, bgate, Wout, bout, pos_bias, offsets=None, **kw):
    x = np.asarray(x, np.float32)
    Wqkv = np.asarray(Wqkv, np.float32)
    bqkv = np.asarray(bqkv, np.float32)
    Wgate = np.asarray(Wgate, np.float32)
    bgate = np.asarray(bgate, np.float32)
    Wout = np.asarray(Wout, np.float32)
    bout = np.asarray(bout, np.float32)
    pos_bias = np.asarray(pos_bias, np.float32)

    if "nc" not in _STATE:
        _STATE["nc"] = _build_nc()
    nc = _STATE["nc"]

    in_maps = _host_prep(x, Wqkv, bqkv, Wgate, bgate, Wout, bout, pos_bias)

    from concourse.bass_utils import run_bass_kernel_spmd

    res = run_bass_kernel_spmd(
        nc, in_maps, core_ids=list(range(NCORES)), **_STATE.get("run_kwargs", {})
    )
    _STATE["last"] = res

    total = np.zeros((N, D), np.float32)
    for r in res.results:
        total += r["out"].astype(np.float32).transpose(1, 0, 2).reshape(N, D)
    total += bout
    return total.reshape(1, N, D).astype(np.float32)


# revision 101
# speedup vs baseline: 1.0009x; 1.0009x over previous
"""Trainium2 Bass kernel for nn_DSQGAttentionN (sparse offset-attention).

Sharding: 16 heads / 8 cores = 2 heads per core (head parallel). Each core
computes its 2 heads' attention plus a column-shard of the gate and output
projection, producing a full-shape (2048, 1024) partial output; the host sums
the 8 partials and adds the output bias.

v4 design:
- Error-compensated fp8 DoubleRow projections: x = x8 + dx8/XDSC, W = W8 + dW8
  (both W terms share the x64 scale), so each projection is 3 DR matmul sets
  (x8*W8 + dx8*W8/XDSC + x8*dW8) with bf16-level accuracy at DR speed.
- V projection emitted directly in [n, j] layout (x chunk stationary, Wv
  moving); bias + softmax-z ones columns ride a rank-1 matmul.
- Dense band offsets 0..192 via 320-wide score windows; exp batched over
  both heads from a [128,2,320] PSUM tile; em mask on DVE/Pool per 4-chunk
  group.
- Dyadic offsets (256..1536): shifted q*k products (validity trimmed),
  ones-matmul reductions, then per-group AV terms as broadcast muls into
  SBUF scratch which the PE accumulates into the group od PSUM via
  identity matmuls (adds run on PE, not DVE). zdy is also matmul-added.
- Pass 2 pipelined one 4-chunk group behind the band; od+dyadic+z live in
  one group PSUM tile (odg); normalization drains on Act with per-partition
  reciprocal scale; out-proj bf16; outsb drains split DVE/Act; DMA out per
  2 chunks.
"""

import math

import numpy as np
import ml_dtypes

BF16 = ml_dtypes.bfloat16
F8 = ml_dtypes.float8_e4m3

N, D, H, HD = 2048, 1024, 16, 64
NCH = 16
NC2 = 4               # DR contraction groups (256 features each)
W = 320
DY6 = [256, 384, 512, 768, 1024, 1536]
NDY = 6
VPAD = 3
NCORES = 8
WSCALE = 64.0         # fp8 weight pre-scale

_DENSE_LOCAL_W = 64
_DYADIC = [96, 128, 192, 256, 384, 512, 768, 1024, 1536]
OFFSETS = sorted(set(range(0, _DENSE_LOCAL_W + 1)) | set(_DYADIC))
BAND_OFFS = sorted(set(range(0, 65)) | {96, 128, 192})

# layout of the packed bf16 tensor (per-partition columns)
_BF_WO = 0
_BF_EM = _BF_WO + D               # [2, W]
_BF_V9 = _BF_EM + 2 * W           # [NCH, NDY, 2]
_BF_END = _BF_V9 + NCH * NDY * 2

_STATE = {}


def _build_nc(debug=False):
    import concourse.bass as bass
    import concourse.tile as tile
    import concourse.mybir as mybir
    from concourse import bacc
    from concourse.bass import ds
    from concourse.masks import make_identity
    from contextlib import ExitStack

    dt = mybir.dt
    f32, bf, f8 = dt.float32, dt.bfloat16, dt.float8e4
    AF = mybir.ActivationFunctionType
    OP = mybir.AluOpType
    AX = mybir.AxisListType
    PM = mybir.MatmulPerfMode

    nc = bacc.Bacc("TRN2")

    # DRAM I/O
    xdr_d = [nc.dram_tensor(f"xdr{c}", (128, 2, N), f8, kind="ExternalInput").ap()
             for c in range(NC2)]
    xde_d = [nc.dram_tensor(f"xde{c}", (128, 2, N), f8, kind="ExternalInput").ap()
             for c in range(NC2)]
    # w8all: [wq, wk, wg, dwq, dwk, dwg] each [NC2,2,128] then [wv, dwv] [NC2,2,130]
    w8_d = nc.dram_tensor("w8all", (128, 6 * NC2 * 2 * 128 + 2 * NC2 * 2 * 130),
                          f8, kind="ExternalInput").ap()
    bfp_d = nc.dram_tensor("bfall", (128, _BF_END), bf, kind="ExternalInput").ap()
    bias_d = nc.dram_tensor("bias4", (128, 4), f32, kind="ExternalInput").ap()
    aux_d = nc.dram_tensor("aux", (1, 384), bf, kind="ExternalInput").ap()
    out = nc.dram_tensor("out", (128, NCH, D), bf, kind="ExternalOutput").ap()
    dbg = {}
    if debug:
        for nm, shp, dtt in [
            ("dqt", (128, N), bf), ("dkt", (128, N), bf),
            ("dgt", (128, N), bf), ("dv2", (128, VPAD + NCH, 130), bf),
            ("dpt", (128, 2, NCH, W), bf), ("dzdy", (128, NCH, 2), bf),
            ("ddyPh", (128, NCH, NDY, 2), bf), ("dfgT", (128, NCH, 128), bf),
            ("dat", (128, 2, 128), bf), ("drz", (128, 2, 2), dt.float32),
            ("dodg", (128, 2, 130), dt.float32),
        ]:
            dbg[nm] = nc.dram_tensor(nm, shp, dtt, kind="ExternalOutput").ap()

    with tile.TileContext(nc) as tc, ExitStack() as ctx:
        sing = ctx.enter_context(tc.tile_pool(name="sing", bufs=1))

        xdr = [sing.tile([128, 2, N], f8, name=f"xdr{c}") for c in range(NC2)]
        xde = [sing.tile([128, 2, N], f8, name=f"xde{c}") for c in range(NC2)]
        w8 = sing.tile([128, 6 * NC2 * 2 * 128 + 2 * NC2 * 2 * 130], f8)
        bfp = sing.tile([128, _BF_END], bf)
        bias_s = sing.tile([128, 4], f32)
        aux_s = sing.tile([1, 384], bf)

        SEG = NC2 * 2 * 128
        wq8 = w8[:, 0 * SEG:1 * SEG].rearrange("p (c r m) -> p c r m", c=NC2, r=2)
        wk8 = w8[:, 1 * SEG:2 * SEG].rearrange("p (c r m) -> p c r m", c=NC2, r=2)
        wg8 = w8[:, 2 * SEG:3 * SEG].rearrange("p (c r m) -> p c r m", c=NC2, r=2)
        dwq8 = w8[:, 3 * SEG:4 * SEG].rearrange("p (c r m) -> p c r m", c=NC2, r=2)
        dwk8 = w8[:, 4 * SEG:5 * SEG].rearrange("p (c r m) -> p c r m", c=NC2, r=2)
        dwg8 = w8[:, 5 * SEG:6 * SEG].rearrange("p (c r m) -> p c r m", c=NC2, r=2)
        SEGV = NC2 * 2 * 130
        wv8 = w8[:, 6 * SEG:6 * SEG + SEGV].rearrange(
            "p (c r m) -> p c r m", c=NC2, r=2)
        dwv8 = w8[:, 6 * SEG + SEGV:6 * SEG + 2 * SEGV].rearrange(
            "p (c r m) -> p c r m", c=NC2, r=2)

        wo_s = bfp[:, _BF_WO:_BF_EM]
        em_s = bfp[:, _BF_EM:_BF_V9].rearrange("p (h w) -> p h w", h=2)
        v9_s = bfp[:, _BF_V9:_BF_END].rearrange(
            "p (c i h) -> p c i h", c=NCH, i=NDY)

        qt = sing.tile([128, N], bf)
        kt = sing.tile([128, N], bf)
        gt = sing.tile([128, N], bf)
        v2i = sing.tile([128, VPAD + NCH, 130], bf)
        prod = sing.tile([128, NDY, N], bf)
        ptall = sing.tile([128, 2, NCH, W], bf)
        dyPh = sing.tile([128, NCH, NDY, 2], bf)
        zdy = sing.tile([128, NCH, 2], bf)
        dyA = sing.tile([128, NCH, 128], bf)
        fgT = sing.tile([128, NCH, 128], bf)
        outsb = sing.tile([128, NCH, D], bf)
        hmask = sing.tile([128, 2], bf)
        ident = sing.tile([128, 128], bf)
        dbg_at = None
        dbg_rz = None
        if debug:
            dbg_at = sing.tile([128, 2, 128], bf, name="dbg_at")
            dbg_rz = sing.tile([128, 2, 2], f32, name="dbg_rz")
            dbg_odg = sing.tile([128, 2, 130], f32, name="dbg_odg")

        bq_c = bias_s[:, 0:1]
        bk_c = bias_s[:, 1:2]
        bg_c = bias_s[:, 2:3]

        bvrow = aux_s[:, 0:130]
        ones1 = aux_s[:, 130:258]

        def ap_of(t, extra, off_elems=0):
            return bass.AP(
                tensor=t.tensor, offset=t.offset + off_elems,
                ap=[list(t.ap[0])] + extra,
            )

        make_identity(nc, ident)
        nc.gpsimd.memset(hmask, 0.0)
        nc.gpsimd.memset(hmask[0:64, 0:1], 1.0)
        nc.gpsimd.memset(hmask[64:128, 1:2], 1.0)
        nc.gpsimd.memset(v2i[:, 0:VPAD, :], 0.0)
        nc.gpsimd.memset(dyA[:, 0:2, :], 0.0)

        # ---- input DMAs (order matters: HWDGE serializes) ----
        nc.sync.dma_start(out=xdr[0], in_=xdr_d[0])
        nc.sync.dma_start(out=w8, in_=w8_d)
        nc.sync.dma_start(out=xde[0], in_=xde_d[0])
        for c2 in range(1, NC2):
            nc.sync.dma_start(out=xdr[c2], in_=xdr_d[c2])
            nc.sync.dma_start(out=xde[c2], in_=xde_d[c2])
        nc.sync.dma_start(out=bias_s, in_=bias_d)
        nc.sync.dma_start(out=aux_s, in_=aux_d)
        nc.sync.dma_start(out=bfp, in_=bfp_d)

        # ================= QK projection (compensated fp8 DR) ============
        with ExitStack() as pctx:
            psQK = pctx.enter_context(
                tc.tile_pool(name="psQK", bufs=8, space="PSUM")
            )
            qk_ps = [
                psQK.tile([128, 512], f32, tag="qk", name=f"qk{i}")
                for i in range(8)
            ]
            for wu in range(30):
                nc.tensor.matmul(
                    qk_ps[7][:, 0:128], ident, ident,
                    start=True, stop=True, skip_group_check=True,
                )
            # all 3 comp sets per c2: PE streams behind the per-c2 DMAs
            for c2 in range(NC2):
                for xs, w_list in ((xdr, (wq8, wk8)), (xdr, (dwq8, dwk8)),
                                   (xde, (wq8, wk8))):
                    for pj, w_sb in enumerate(w_list):
                        for b in range(4):
                            nc.tensor.matmul(
                                qk_ps[pj * 4 + b], w_sb[:, c2],
                                xs[c2][:, :, ds(b * 512, 512)],
                                start=(c2 == 0 and xs is xdr
                                       and w_sb in (wq8, wk8)
                                       and w_list[0] is wq8),
                                stop=(c2 == NC2 - 1 and xs is xde),
                                perf_mode=PM.DoubleRow,
                            )
            sq = 1.0 / (WSCALE * 8.0)
            sk = 1.0 / WSCALE
            nc.scalar.activation(qt[:, ds(0, 512)], qk_ps[0], AF.Identity,
                                 bias=bq_c, scale=sq)
            nc.vector.tensor_scalar(kt[:, ds(0, 512)], qk_ps[4],
                                    sk, bk_c, OP.mult, OP.add)
            nc.scalar.activation(qt[:, ds(512, 512)], qk_ps[1], AF.Identity,
                                 bias=bq_c, scale=sq)
            nc.vector.tensor_scalar(kt[:, ds(512, 512)], qk_ps[5],
                                    sk, bk_c, OP.mult, OP.add)
            nc.scalar.activation(qt[:, ds(1024, 512)], qk_ps[2], AF.Identity,
                                 bias=bq_c, scale=sq)
            nc.vector.tensor_scalar(kt[:, ds(1024, 512)], qk_ps[6],
                                    sk, bk_c, OP.mult, OP.add)
            nc.vector.tensor_scalar(qt[:, ds(1536, 512)], qk_ps[3],
                                    sq, bq_c, OP.mult, OP.add)
            nc.scalar.activation(kt[:, ds(1536, 512)], qk_ps[7], AF.Identity,
                                 bias=bk_c, scale=sk)

        # ================= middle phase =================
        with ExitStack() as mctx:
            psDY = mctx.enter_context(
                tc.tile_pool(name="psDY", bufs=1, space="PSUM"))
            sbp = mctx.enter_context(tc.tile_pool(name="sbp", bufs=2))

            dy_ps = psDY.tile([128, NCH, NDY * 2], f32, name="dy_ps")
            nc.vector.memset(dy_ps, 0.0)

            def emit_prod(i):
                d = DY6[i]
                nc.vector.tensor_mul(
                    prod[:, i, d:N], qt[:, d:N], kt[:, 0:N - d]
                )

            def emit_reduce(i):
                m = DY6[i] // 128
                for c in range(m, NCH):
                    o1 = bass.AP(
                        tensor=dy_ps.tensor,
                        offset=dy_ps.offset + c * NDY * 2 + i * 2,
                        ap=[list(dy_ps.ap[0]), [1, 2]],
                    )
                    nc.tensor.matmul(
                        o1, prod[:, i, ds(c * 128, 128)], hmask,
                        start=True, stop=True, skip_group_check=True,
                    )

            odg_tiles = {}

            def valid_pair(p):
                return [i for i in range(NDY) if DY6[i] // 128 <= 2 * p + 1]

            def emit_od_pair(p):
                odg = psO.tile([128, 2, 130], f32, tag="odg", name="odg")
                odg_tiles[p] = odg
                for u in range(2):
                    qc = 2 * p + u
                    nsub = 1 + (qc >= 1) + (qc >= 2)
                    for hl in range(2):
                        mv = ap_of(v2i, [[2, 65]], (VPAD + qc) * 130 + hl)
                        nc.tensor.matmul(
                            ap_of(odg, [[1, 65]], u * 130 + hl * 65),
                            ptall[:, hl, qc, 0:128], mv,
                            start=True, stop=(nsub == 1),
                            skip_group_check=True,
                        )
                        if qc >= 1:
                            mv = ap_of(v2i, [[2, 65]],
                                       (VPAD + qc - 1) * 130 + hl)
                            nc.tensor.matmul(
                                ap_of(odg, [[1, 65]], u * 130 + hl * 65),
                                ptall[:, hl, qc - 1, 128:256], mv,
                                start=False, stop=(nsub == 2),
                                skip_group_check=True,
                            )
                        if qc >= 2:
                            mv = ap_of(v2i, [[2, 65]],
                                       (VPAD + qc - 2) * 130 + hl)
                            nc.tensor.matmul(
                                bass.AP(
                                    tensor=odg.tensor,
                                    offset=odg.offset + u * 130 + hl * 65,
                                    ap=[[list(odg.ap[0])[0], 64], [1, 65]],
                                ),
                                ptall[:, hl, qc - 2, 256:W], mv,
                                start=False, stop=True,
                                skip_group_check=True,
                            )
            def av_pair(p):
                # dyadic AV accumulation into dyA (DVE for late pairs, Pool
                # for early ones which land while DVE is still busy)
                valid = valid_pair(p)
                eng2 = nc.gpsimd if p <= 3 else nc.vector

                def acc_v():
                    return ap_of(dyA, [[128, 2], [2, 64], [1, 2]],
                                 2 * p * 128)

                def src_v(i):
                    m = DY6[i] // 128
                    return ap_of(v2i, [[130, 2], [2, 64], [1, 2]],
                                 (VPAD - m + 2 * p) * 130)

                def alpha_v(i):
                    return ap_of(dyPh, [[NDY * 2, 2], [0, 64], [1, 2]],
                                 2 * p * NDY * 2 + i * 2)

                eng3 = eng2
                if valid:
                    eng2.tensor_mul(acc_v(), src_v(valid[0]),
                                    alpha_v(valid[0]))
                    for i in valid[1:]:
                        tmp = sbp.tile([128, 2, 128], bf, tag="avt",
                                       name="avt", bufs=4)
                        tv = ap_of(tmp, [[128, 2], [2, 64], [1, 2]])
                        eng2.tensor_mul(tv, src_v(i), alpha_v(i))
                        eng3.tensor_add(acc_v(), acc_v(), tv)

            at_tiles = {}

            ods_tiles = {}

            def emit_ods(p):
                odg = odg_tiles.pop(p)
                if debug and p == 0:
                    nc.vector.tensor_copy(dbg_odg, odg)
                ods = sbf.tile([128, 2, 130], bf, tag="ods", name="ods",
                               bufs=4)
                nc.scalar.copy(ods, odg)
                ods_tiles[p] = (ods, odg)

            def pass2_stage1(p):
                ods, odg = ods_tiles.pop(p)
                ats = []
                for u in range(2):
                    qc = 2 * p + u
                    zt = sbf.tile([128, 2], f32, tag="zt", name="zt")
                    nc.vector.tensor_add(
                        zt, ap_of(odg, [[65, 2]], u * 130 + 64),
                        zdy[:, qc, :])
                    rzt = sbf.tile([128, 2], f32, tag="rzt", name="rzt")
                    nc.vector.reciprocal(rzt, zt)
                    a_t = sbf.tile([128, 128], bf, tag="a", name="a", bufs=16)
                    av = ap_of(a_t, [[2, 64], [1, 2]])
                    nc.vector.tensor_add(
                        av,
                        ap_of(ods, [[1, 64], [65, 2]], u * 130),
                        ap_of(dyA, [[2, 64], [1, 2]], qc * 128),
                    )
                    for hl in range(2):
                        nc.vector.tensor_scalar_mul(
                            ap_of(a_t, [[2, 64]], hl),
                            ap_of(a_t, [[2, 64]], hl),
                            rzt[:, hl:hl + 1],
                        )
                    if debug and qc in (0, 1):
                        nc.vector.tensor_copy(dbg_at[:, qc, :], a_t)
                        nc.vector.tensor_copy(dbg_rz[:, qc, :], rzt)
                    ats.append(a_t)
                at_tiles[p] = ats


            # ---- scope A: projections + bands + dyadic prep ----
            with ExitStack() as vctx:
                psV = vctx.enter_context(
                    tc.tile_pool(name="psV", bufs=2, space="PSUM"))
                psG = vctx.enter_context(
                    tc.tile_pool(name="psG", bufs=2, space="PSUM"))
                psS = vctx.enter_context(
                    tc.tile_pool(name="psS", bufs=2, space="PSUM"))

                def band_kc(kc):
                    span = min(W, N - 128 * kc)
                    for hl in range(2):
                        st = psS.tile([128, W], f32, tag="st", name="st")
                        hp = ds(64 * hl, 64)
                        nc.tensor.matmul(
                            st[:, 0:span],
                            kt[hp, ds(kc * 128, 128)],
                            qt[hp, ds(kc * 128, span)],
                            start=True, stop=True, skip_group_check=True,
                        )
                        nc.scalar.activation(
                            ptall[:, hl, kc, 0:span], st[:, 0:span], AF.Exp
                        )

                def band_mask(g):
                    o = ap_of(ptall, [[NCH * W, 2], [W, 4], [1, W]], g * 4 * W)
                    e = ap_of(em_s, [[W, 2], [0, 4], [1, W]])
                    nc.vector.tensor_mul(o, o, e)

                def dyadic_group(g):
                    g4 = ds(g * 4, 4)
                    nc.scalar.activation(dyPh[:, g4, :, :], dy_ps[:, g4, :],
                                         AF.Exp)
                    eng = nc.gpsimd if g == 0 else nc.vector
                    eng.tensor_mul(dyPh[:, g4, :, :], dyPh[:, g4, :, :],
                                   v9_s[:, g4, :, :])
                    with nc.allow_low_precision(reason="z of <=6 bf16 terms"):
                        nc.vector.tensor_reduce(
                            zdy[:, g4, :],
                            dyPh[:, g4, :, :].rearrange("p c i h -> p c h i"),
                            AX.X, OP.add,
                        )

                def v_pair(c0):
                    ps = psV.tile([128, 2, 130], f32, tag="v", name="v")
                    for u in range(2):
                        c = c0 + u
                        for c2 in range(NC2):
                            nc.tensor.matmul(
                                ps[:, u], xdr[c2][:, :, ds(c * 128, 128)],
                                wv8[:, c2],
                                start=(c2 == 0), stop=False,
                                perf_mode=PM.DoubleRow,
                                skip_group_check=True,
                            )
                        for c2 in range(NC2):
                            nc.tensor.matmul(
                                ps[:, u], xdr[c2][:, :, ds(c * 128, 128)],
                                dwv8[:, c2],
                                start=False, stop=False,
                                perf_mode=PM.DoubleRow,
                                skip_group_check=True,
                            )
                        for c2 in range(NC2):
                            nc.tensor.matmul(
                                ps[:, u], xde[c2][:, :, ds(c * 128, 128)],
                                wv8[:, c2],
                                start=False, stop=False,
                                perf_mode=PM.DoubleRow,
                                skip_group_check=True,
                            )
                        nc.tensor.matmul(ps[:, u], ones1, bvrow,
                                         start=False, stop=True,
                                         skip_group_check=True)
                    nc.vector.tensor_scalar_mul(
                        v2i[:, VPAD + c0:VPAD + c0 + 2, :], ps,
                        1.0 / WSCALE)

                def g_block(b):
                    ps = psG.tile([128, 512], f32, tag="g", name="g")
                    for c2 in range(NC2):
                        nc.tensor.matmul(
                            ps, wg8[:, c2], xdr[c2][:, :, ds(b * 512, 512)],
                            start=(c2 == 0), stop=False,
                            perf_mode=PM.DoubleRow,
                        )
                    for c2 in range(NC2):
                        nc.tensor.matmul(
                            ps, dwg8[:, c2], xdr[c2][:, :, ds(b * 512, 512)],
                            start=False, stop=False,
                            perf_mode=PM.DoubleRow,
                        )
                    for c2 in range(NC2):
                        nc.tensor.matmul(
                            ps, wg8[:, c2], xde[c2][:, :, ds(b * 512, 512)],
                            start=False, stop=(c2 == NC2 - 1),
                            perf_mode=PM.DoubleRow,
                        )
                    nc.scalar.activation(gt[:, ds(b * 512, 512)], ps, AF.Tanh,
                                         bias=bg_c, scale=0.5 / WSCALE)

                for kc in range(0, 4):
                    band_kc(kc)
                for c0 in range(0, 8, 2):
                    v_pair(c0)
                for kc in range(4, 8):
                    band_kc(kc)
                emit_prod(5)
                emit_prod(4)
                for kc in range(8, 12):
                    band_kc(kc)
                emit_prod(3)
                for c0 in range(8, NCH, 2):
                    v_pair(c0)
                emit_prod(2)
                for kc in range(12, NCH):
                    band_kc(kc)
                emit_prod(1)
                g_block(0)
                emit_prod(0)
                emit_reduce(5)
                g_block(1)
                emit_reduce(4)
                emit_reduce(3)
                band_mask(0)
                g_block(2)
                emit_reduce(2)
                band_mask(1)
                g_block(3)
                emit_reduce(1)
                emit_reduce(0)
                for b in range(4):
                    nc.vector.tensor_scalar(
                        gt[:, ds(b * 512, 512)], gt[:, ds(b * 512, 512)],
                        0.5, 0.5, OP.mult, OP.add,
                    )
                band_mask(2)
                band_mask(3)
                for g in range(4):
                    dyadic_group(g)
            # ---- scope B: od + pass 2 ----
            psO = mctx.enter_context(
                tc.tile_pool(name="psO", bufs=2, space="PSUM"))
            psF = mctx.enter_context(
                tc.tile_pool(name="psF", bufs=2, space="PSUM"))
            psQ = mctx.enter_context(
                tc.tile_pool(name="psQ", bufs=3, space="PSUM"))
            sbf = mctx.enter_context(tc.tile_pool(name="sbf", bufs=4))

            def pass2_stage2(p):
                ats = at_tiles.pop(p)
                for u in range(2):
                    qc = 2 * p + u
                    a_t = ats[u]
                    tp2 = psF.tile([128, 128], bf, tag="tp2", name="tp2")
                    nc.tensor.transpose(tp2, a_t, ident)
                    nc.vector.tensor_mul(fgT[:, qc, :], tp2,
                                         gt[:, ds(qc * 128, 128)])
                    for jb in range(2):
                        po = psQ.tile([128, 512], f32, tag="po", name="po")
                        nc.tensor.matmul(po, fgT[:, qc, :],
                                         wo_s[:, ds(jb * 512, 512)],
                                         start=True, stop=True)
                        osl = outsb[:, qc, ds(jb * 512, 512)]
                        if (2 * qc + jb) % 8 < 7:
                            nc.scalar.copy(osl, po)
                        else:
                            nc.vector.tensor_copy(osl, po)
                    nc.sync.dma_start(
                        out=out[:, qc:qc + 1, :],
                        in_=outsb[:, qc:qc + 1, :],
                    )

            for p in range(8):
                if p >= 1:
                    pass2_stage2(p - 1)
                emit_od_pair(p)
                av_pair(p)
                emit_ods(p)
                pass2_stage1(p)
            pass2_stage2(7)

            if debug:
                for nm, sb in [("dqt", qt), ("dkt", kt), ("dgt", gt),
                               ("dv2", v2i), ("dpt", ptall), ("dzdy", zdy),
                               ("ddyPh", dyPh), ("dfgT", fgT),
                               ("dat", dbg_at), ("drz", dbg_rz),
                               ("dodg", dbg_odg)]:
                    if nm in dbg:
                        nc.sync.dma_start(out=dbg[nm], in_=sb)

    nc.compile()
    return nc


def _host_prep(x, Wqkv, bqkv, Wgate, bgate, Wout, bout, pos_bias):
    x2 = np.asarray(x, np.float32).reshape(N, D)
    # xdr[p, c2, r, n] = x[n, 256*c2 + 128*r + p]
    xT = np.ascontiguousarray(x2.T).reshape(NC2, 2, 128, N)
    xTp = np.ascontiguousarray(xT.transpose(2, 0, 1, 3))
    xdr = xTp.astype(F8)
    xde = (xTp - xdr.astype(np.float32)).astype(F8)
    xmap = {}
    for c in range(NC2):
        xmap[f"xdr{c}"] = np.ascontiguousarray(xdr[:, c])
        xmap[f"xde{c}"] = np.ascontiguousarray(xde[:, c])

    psi = np.array([64 * (j % 2) + j // 2 for j in range(128)])

    def wpack8(Wcols):
        # Wcols: (1024, ncol) f32 *WSCALE -> base fp8 [p, c2, r, m] and delta
        ncol = Wcols.shape[1]
        Wt = np.ascontiguousarray(Wcols * WSCALE).reshape(NC2, 2, 128, ncol)
        Wt = np.ascontiguousarray(Wt.transpose(2, 0, 1, 3))
        w0 = Wt.astype(F8)
        dw = (Wt - w0.astype(np.float32)).astype(F8)
        return w0, dw

    in_maps = []
    for cid in range(NCORES):
        r0 = 128 * cid
        wq, dwq = wpack8(Wqkv[r0:r0 + 128].T)
        wk, dwk = wpack8(Wqkv[D + r0:D + r0 + 128].T)
        wgp, dwg = wpack8(Wgate[r0:r0 + 128][psi].T)
        wv130 = np.zeros((D, 130), np.float32)
        wv130[:, 0:128] = Wqkv[2 * D + r0:2 * D + r0 + 128][psi].T
        wv, dwv = wpack8(wv130)

        w8all = np.concatenate(
            [t.reshape(128, -1) for t in (wq, wk, wgp, dwq, dwk, dwg, wv, dwv)],
            axis=1,
        )

        wop = np.ascontiguousarray(Wout[:, r0:r0 + 128][:, psi].T)  # (128,1024)

        em = np.zeros((128, 2, W), np.float32)
        p = np.arange(128)[:, None]
        j = np.arange(W)[None, :]
        o = j - p
        sel = np.isin(o, BAND_OFFS)
        oc = np.clip(o, 0, 192)
        for hl in range(2):
            eb = np.array(
                [math.exp(pos_bias[OFFSETS.index(v), 2 * cid + hl])
                 if v in BAND_OFFS else 0.0 for v in range(193)],
                np.float32,
            )
            em[:, hl, :] = np.where(sel, eb[oc], 0.0)

        nidx = (np.arange(128)[:, None, None, None]
                + 128 * np.arange(NCH)[None, :, None, None])
        dys = np.array(DY6)[None, None, :, None]
        valid = (nidx >= dys).astype(np.float32)
        eb9 = np.zeros((1, 1, NDY, 2), np.float32)
        for hl in range(2):
            for i, d in enumerate(DY6):
                eb9[0, 0, i, hl] = math.exp(
                    pos_bias[OFFSETS.index(d), 2 * cid + hl])
        v9t = valid * eb9

        bfall = np.concatenate(
            [wop, em.reshape(128, -1), v9t.reshape(128, -1)], axis=1,
        ).astype(BF16)

        biases = np.stack(
            [
                bqkv[r0:r0 + 128] * 0.125,
                bqkv[D + r0:D + r0 + 128],
                0.5 * bgate[r0:r0 + 128][psi],
                np.zeros(128, np.float32),
            ],
            axis=1,
        ).astype(np.float32)

        aux = np.zeros((1, 384), np.float32)
        aux[0, 0:128] = bqkv[2 * D + r0:2 * D + r0 + 128][psi] * WSCALE
        aux[0, 128:130] = WSCALE
        aux[0, 130:258] = 1.0

        in_maps.append({
            **xmap,
            "w8all": np.ascontiguousarray(w8all),
            "bfall": np.ascontiguousarray(bfall),
            "bias4": np.ascontiguousarray(biases),
            "aux": np.ascontiguousarray(aux).astype(BF16),
        })
    return in_maps


def kernel(x, Wqkv, bqkv, Wgate, bgate, Wout, bout, pos_bias, offsets=None, **kw):
    x = np.asarray(x, np.float32)
    Wqkv = np.asarray(Wqkv, np.float32)
    bqkv = np.asarray(bqkv, np.float32)
    Wgate = np.asarray(Wgate, np.float32)
    bgate = np.asarray(bgate, np.float32)
    Wout = np.asarray(Wout, np.float32)
    bout = np.asarray(bout, np.float32)
    pos_bias = np.asarray(pos_bias, np.float32)

    if "nc" not in _STATE:
        _STATE["nc"] = _build_nc()
    nc = _STATE["nc"]

    in_maps = _host_prep(x, Wqkv, bqkv, Wgate, bgate, Wout, bout, pos_bias)

    from concourse.bass_utils import run_bass_kernel_spmd

    res = run_bass_kernel_spmd(
        nc, in_maps, core_ids=list(range(NCORES)), **_STATE.get("run_kwargs", {})
    )
    _STATE["last"] = res

    total = np.zeros((N, D), np.float32)
    for r in res.results:
        total += r["out"].astype(np.float32).transpose(1, 0, 2).reshape(N, D)
    total += bout
    return total.reshape(1, N, D).astype(np.float32)
